# revision 11
# baseline (speedup 1.0000x reference)
"""Trainium2 Bass kernel for nn_CrossAttention (cross-attention + MLP block).

Sharding: 8 cores = batch B(4) x query-halves (2x512 rows). Each core computes
its (batch, m-slice); K/V projections for a batch are duplicated across its
pair core (no collectives).

Layouts (T = features on partitions, tokens on free dim):
  kT [e, n] f32r; vp [n, e] bf16 interleaved per-head with a ones column;
  qT [e, m] f32r.
  scores path A [m, n] -> exp on ACT (+row-sum accum_out) -> attn HBM output.
  scores path B [n, m] -> exp bf16 -> ctx matmul with ones-augmented vp:
    psum rows 0-63 = ctx^T (unnormalized), row 64 = softmax Z.
  out_proj [m, e] (ctxT stationary, bf16); resid+LN in [m, d]; PE-transpose
  ln -> lnT; MLP in T layout f32r; linear2 emits [m, d] for final residual.
"""

import numpy as np
import ml_dtypes

import concourse.bass as bass
import concourse.tile as tile
from concourse import bacc, mybir
from concourse.bass_utils import run_bass_kernel_spmd
from concourse.masks import make_identity

F32 = mybir.dt.float32
F32R = mybir.dt.float32r
BF16 = mybir.dt.bfloat16
AF = mybir.ActivationFunctionType
ALU = mybir.AluOpType

B, M_FULL, N, D, H, HD, MLP = 4, 1024, 2048, 1024, 16, 64, 4096
EPS = 1e-5
N_CORES = 8
M = M_FULL // 2          # 512 query rows per core
DT = D // 128            # 8 d-tiles
ET = D // 128            # 8 e-tiles
FT = MLP // 128          # 32 f-tiles
NT = N // 128            # 16 n-tiles
MT = M // 128            # 4 m-tiles
NC4 = N // 512           # 4 n-chunks of 512

_CACHE = {}


def _build():
    nc = bacc.Bacc("TRN2", target_bir_lowering=False, debug=False,
                   num_devices=N_CORES)

    def din(name, shape, dt=F32R):
        return nc.dram_tensor(name, shape, dt, kind="ExternalInput").ap()

    xt = din("xt", [128, DT, N])              # x[b].T tiled  [p, dt, n]
    qt = din("qt", [128, DT, M])              # q-slice.T tiled
    q_in = din("q_in", [M, D], F32)           # q-slice (natural layout)
    wqt = din("wqt", [ET, 128, DT, 128])      # w_q.T tiled per e-tile
    wkt = din("wkt", [ET, 128, DT, 128])
    wvt = din("wvt", [2, 128, DT, 512])       # w_v.T tiled per e-chunk
    wot = din("wot", [2, 128, DT, 512], BF16)  # w_o.T tiled per e-chunk
    w1t = din("w1t", [FT, 128, DT, 128])      # (w1*g).T tiled per f-tile
    w2t = din("w2t", [FT, 128, D])            # w2.T tiled per f-tile
    bq = din("bq", [128, ET], F32)            # b_q per e-tile (partition-major)
    bk = din("bk", [128, ET], F32)
    bv = din("bv", [1, D], F32)
    bo = din("bo", [1, D], F32)
    b1 = din("b1", [128, FT], F32)            # b1 + w1@ln2_b, per f-tile
    b2 = din("b2", [1, D], F32)

    attn_out = nc.dram_tensor("attn_out", [H, M, N], F32,
                              kind="ExternalOutput").ap()
    out_out = nc.dram_tensor("out_out", [M, D], F32, kind="ExternalOutput").ap()

    with tile.TileContext(nc) as tc:
        with (
            tc.tile_pool(name="pp", bufs=1) as pp,          # small persistents
            tc.tile_pool(name="pmid", bufs=1) as pm,        # ctxT
        ):
            bq_sb = pp.tile([128, ET], F32, tag="bq")
            nc.sync.dma_start(bq_sb[:], bq)
            bk_sb = pp.tile([128, ET], F32, tag="bk")
            nc.sync.dma_start(bk_sb[:], bk)
            ctxT = pm.tile([128, ET, M], BF16, tag="ctxT")

            with (
                tc.tile_pool(name="pstage1", bufs=1) as s1,     # xt
                tc.tile_pool(name="pattn", bufs=1) as pa_sb,    # qts, vp
                tc.tile_pool(name="kpool", bufs=2) as kp,       # kT rotating
                tc.tile_pool(name="wqk", bufs=1) as wqk,
                tc.tile_pool(name="wvpool", bufs=1) as wvp,
                tc.tile_pool(name="expp", bufs=2) as ab,        # exp [m,n] tiles
                tc.tile_pool(name="etp", bufs=4) as etp,
                tc.tile_pool(name="rzp", bufs=2) as rzp,
                tc.tile_pool(name="rzd", bufs=2, space="DRAM") as rzd,        # expT bf16 tiles
                tc.tile_pool(name="small", bufs=4) as sm,
                tc.tile_pool(name="psproj", bufs=1, space="PSUM") as ps_proj,
                tc.tile_pool(name="psA", bufs=2, space="PSUM") as ps_a,
                tc.tile_pool(name="psB", bufs=2, space="PSUM") as ps_b,
                tc.tile_pool(name="psctx", bufs=1, space="PSUM") as ps_ctx,
            ):
                xt_sb = s1.tile([128, DT, N], F32R, tag="xt")
                nc.sync.dma_start(xt_sb[:], xt)
                qt_sb = s1.tile([128, DT, M], F32R, tag="qt")
                nc.sync.dma_start(qt_sb[:], qt)

                qts = pa_sb.tile([128, ET, M], F32R, tag="qts")
                # vp for one e-chunk (8 heads): [n-part, nt, head, 64+ones]
                vp_sb = pa_sb.tile([128, NT, 8, HD + 1], BF16, tag="vp")

                # ---- Q projection ----
                for et in range(ET):
                    w = wqk.tile([128, DT, 128], F32R, tag="wq")
                    nc.sync.dma_start(w[:], wqt[et])
                    p = ps_proj.tile([128, 512], F32, tag="pp")
                    for dt_i in range(DT):
                        nc.tensor.matmul(p[:], w[:, dt_i, :], qt_sb[:, dt_i, :],
                                         start=(dt_i == 0), stop=(dt_i == DT - 1))
                    nc.vector.tensor_scalar_add(qts[:, et, :], p[:],
                                                bq_sb[:, et:et + 1])

                # ---- V projection for e-chunk ec into vp_sb ----
                def v_proj(ec):
                    nc.vector.memset(vp_sb[:, :, :, HD], 1.0)
                    wv = wvp.tile([128, DT, 512], F32R, tag="wv")
                    nc.sync.dma_start(wv[:], wvt[ec])
                    bvb_t = wvp.tile([128, 512], F32, tag="bvb")
                    nc.gpsimd.dma_start(
                        bvb_t[:],
                        bv[0:1, ec * 512:(ec + 1) * 512].to_broadcast((128, 512)))
                    bvb = bvb_t[:].rearrange("p (h e) -> p h e", e=HD)
                    for nt in range(NT):
                        p = ps_proj.tile([128, 512], F32, tag="pp")
                        for dt_i in range(DT):
                            nc.tensor.matmul(
                                p[:], xt_sb[:, dt_i, nt * 128:(nt + 1) * 128],
                                wv[:, dt_i, :],
                                start=(dt_i == 0), stop=(dt_i == DT - 1))
                        nc.vector.tensor_tensor(
                            out=vp_sb[:, nt, :, 0:HD],
                            in0=p[:].rearrange("p (h e) -> p h e", e=HD),
                            in1=bvb, op=ALU.add)

                v_proj(0)

                # ---- per e-tile: K projection + attention for its 2 heads ----
                def head_attention(et, kt, hh):
                    h = 2 * et + hh
                    hs = slice(hh * HD, (hh + 1) * HD)
                    # ---- path A: scores [m, n] -> exp -> attn out ----
                    for mt in range(MT):
                        qa = qts[hs, et, mt * 128:(mt + 1) * 128]
                        exp_s = ab.tile([128, N], F32, tag="exps")
                        zs = []
                        for half in range(2):
                            pa = ps_a.tile([128, 1024], F32, tag="pa")
                            for j in range(2):
                                nc4 = half * 2 + j
                                nc.tensor.matmul(
                                    pa[:, j * 512:(j + 1) * 512], qa,
                                    kt[hs, nc4 * 512:(nc4 + 1) * 512],
                                    start=True, stop=True)
                            z = sm.tile([128, 1], F32, tag="z")
                            nc.scalar.activation(
                                out=exp_s[:, half * 1024:(half + 1) * 1024],
                                in_=pa[:], func=AF.Exp, scale=0.125,
                                accum_out=z[:])
                            zs.append(z)
                        zt = sm.tile([128, 1], F32, tag="zt")
                        nc.vector.tensor_tensor(out=zt[:], in0=zs[0][:],
                                                in1=zs[1][:], op=ALU.add)
                        rz = sm.tile([128, 1], F32, tag="rz")
                        nc.vector.reciprocal(rz[:], zt[:])
                        nc.vector.tensor_scalar_mul(exp_s[:], exp_s[:], rz[:])
                        nc.sync.dma_start(
                            attn_out[h, mt * 128:(mt + 1) * 128, :], exp_s[:])

                    # ---- path B: scoresT [n, m] -> exp bf16 -> ctx ----
                    pc = ps_ctx.tile([128, M], F32, tag="pc")
                    for nt in range(NT):
                        pb = ps_b.tile([128, M], F32, tag="pb")
                        nc.tensor.matmul(pb[:], kt[hs, nt * 128:(nt + 1) * 128],
                                         qts[hs, et, :], start=True, stop=True)
                        eT = etp.tile([128, M], BF16, tag="eT")
                        nc.scalar.activation(out=eT[:], in_=pb[:], func=AF.Exp,
                                             scale=0.125)
                        nc.tensor.matmul(pc[0:HD + 1, :], vp_sb[:, nt, h % 8, :],
                                         eT[:],
                                         start=(nt == 0), stop=(nt == NT - 1))
                    rzb = sm.tile([1, M], F32, tag="rzb")
                    nc.vector.reciprocal(rzb[:], pc[HD:HD + 1, :])
                    rzdt = rzd.tile([1, M], F32, tag="rzdt")
                    nc.gpsimd.dma_start(rzdt[:], rzb[:])
                    rzb_bc = rzp.tile([HD, M], F32, tag="rzbbc")
                    nc.gpsimd.dma_start(rzb_bc[:],
                                        rzdt[:].to_broadcast((HD, M)))
                    nc.vector.tensor_tensor(
                        out=ctxT[hs, et, :], in0=pc[0:HD, :],
                        in1=rzb_bc[:], op=ALU.mult)

                for et in range(ET):
                    if et == ET // 2:
                        v_proj(1)
                    wk = wqk.tile([128, DT, 128], F32R, tag="wk")
                    nc.sync.dma_start(wk[:], wkt[et])
                    kt = kp.tile([128, N], F32R, tag="kt")
                    for nc4 in range(NC4):
                        p = ps_proj.tile([128, 512], F32, tag="pp")
                        for dt_i in range(DT):
                            nc.tensor.matmul(
                                p[:], wk[:, dt_i, :],
                                xt_sb[:, dt_i, nc4 * 512:(nc4 + 1) * 512],
                                start=(dt_i == 0), stop=(dt_i == DT - 1))
                        nc.vector.tensor_scalar_add(
                            kt[:, nc4 * 512:(nc4 + 1) * 512], p[:],
                            bk_sb[:, et:et + 1])

                    for hh in range(2):
                        head_attention(et, kt, hh)

            # ================= tail =================
            with tc.tile_pool(name="ptail", bufs=1) as tl:
                resid = tl.tile([128, MT, D], F32, tag="resid")
                q_sb = tl.tile([128, MT, D], F32, tag="qin")
                nc.sync.dma_start(q_sb[:], q_in.rearrange("(mt p) d -> p mt d", p=128))
                bo_bc = tl.tile([128, D], F32, tag="bobc")
                nc.gpsimd.dma_start(bo_bc[:], bo[0:1, :].to_broadcast((128, D)))
                b2_bc = tl.tile([128, D], F32, tag="b2bc")
                nc.gpsimd.dma_start(b2_bc[:], b2[0:1, :].to_broadcast((128, D)))
                b1_sb = tl.tile([128, FT], F32, tag="b1")
                nc.sync.dma_start(b1_sb[:], b1)
                ident = tl.tile([128, 128], F32, tag="ident")
                make_identity(nc, ident[:])
                eps_sb = tl.tile([128, 1], F32, tag="eps")
                nc.vector.memset(eps_sb[:], EPS)
                lnT = tl.tile([128, DT, M], F32R, tag="lnT")

                with (
                    tc.tile_pool(name="wo_p", bufs=1) as wop,
                    tc.tile_pool(name="lntmp", bufs=2) as lntmp,
                    tc.tile_pool(name="smt", bufs=4) as smt,
                    tc.tile_pool(name="psop", bufs=2, space="PSUM") as ps_op,
                    tc.tile_pool(name="pstp", bufs=2, space="PSUM") as ps_tp,
                ):
                    wo_sb = wop.tile([128, 2, DT, 512], BF16, tag="wo")
                    nc.sync.dma_start(wo_sb[:], wot.rearrange("c p d e -> p c d e"))
                    # ---- out_proj [m, e] + q + b_o -> resid; LN -> lnT ----
                    for mt in range(MT):
                        for ec in range(2):
                            p = ps_op.tile([128, 512], F32, tag="po")
                            for et in range(ET):
                                nc.tensor.matmul(
                                    p[:], ctxT[:, et, mt * 128:(mt + 1) * 128],
                                    wo_sb[:, ec, et, :],
                                    start=(et == 0), stop=(et == ET - 1))
                            sl = slice(ec * 512, (ec + 1) * 512)
                            nc.vector.tensor_tensor(out=resid[:, mt, sl], in0=p[:],
                                                    in1=q_sb[:, mt, sl], op=ALU.add)
                            nc.vector.tensor_tensor(
                                out=resid[:, mt, sl], in0=resid[:, mt, sl],
                                in1=bo_bc[:, sl],
                                op=ALU.add)
                        # LN stats over free dim (d)
                        stats = smt.tile([128, 2, 6], F32, tag="stats")
                        for g in range(2):
                            nc.vector.bn_stats(
                                out=stats[:, g, :],
                                in_=resid[:, mt, g * 512:(g + 1) * 512])
                        mv = smt.tile([128, 2], F32, tag="mv")
                        nc.vector.bn_aggr(out=mv[:], in_=stats[:])
                        rstd = smt.tile([128, 1], F32, tag="rstd")
                        nc.scalar.activation(out=rstd[:], in_=mv[:, 1:2],
                                             func=AF.Sqrt, bias=eps_sb[:], scale=1.0)
                        nc.vector.reciprocal(rstd[:], rstd[:])
                        nmr = smt.tile([128, 1], F32, tag="nmr")
                        nc.vector.scalar_tensor_tensor(
                            out=nmr[:], in0=mv[:, 0:1], scalar=-1.0, in1=rstd[:],
                            op0=ALU.mult, op1=ALU.mult)
                        ln_t = lntmp.tile([128, D], F32, tag="ln")
                        nc.scalar.activation(out=ln_t[:], in_=resid[:, mt, :],
                                             func=AF.Identity, bias=nmr[:], scale=rstd[:])
                        for dt_i in range(DT):
                            pt = ps_tp.tile([128, 128], F32, tag="pt")
                            nc.tensor.transpose(
                                pt[:], ln_t[:, dt_i * 128:(dt_i + 1) * 128], ident[:])
                            nc.vector.tensor_copy(
                                lnT[:, dt_i, mt * 128:(mt + 1) * 128], pt[:])

                # ---- MLP ----
                h1T = tl.tile([128, FT, M], F32R, tag="h1T")
                with (
                    tc.tile_pool(name="w1p", bufs=2) as w1p,
                    tc.tile_pool(name="gtmp", bufs=2) as gtmp,
                    tc.tile_pool(name="psl1", bufs=2, space="PSUM") as ps_l1,
                ):
                    for ft in range(FT):
                        w = w1p.tile([128, DT, 128], F32R, tag="w1")
                        nc.sync.dma_start(w[:], w1t[ft])
                        p = ps_l1.tile([128, M], F32, tag="pl1")
                        for dt_i in range(DT):
                            nc.tensor.matmul(p[:], w[:, dt_i, :], lnT[:, dt_i, :],
                                             start=(dt_i == 0), stop=(dt_i == DT - 1))
                        g = gtmp.tile([128, M], F32, tag="g")
                        nc.scalar.activation(out=g[:], in_=p[:], func=AF.Gelu,
                                             bias=b1_sb[:, ft:ft + 1], scale=1.0)
                        nc.vector.tensor_copy(h1T[:, ft, :], g[:])

                with (
                    tc.tile_pool(name="w2p", bufs=3) as w2p,
                    tc.tile_pool(name="otmp", bufs=2) as otmp,
                    tc.tile_pool(name="psl2", bufs=8, space="PSUM") as ps_l2,
                ):
                    pl2 = []
                    for _pi in range(2 * MT):
                        pl2_t = ps_l2.tile([128, 512], F32, tag="pl2")
                        pl2.append(pl2_t)
                    for ft in range(FT):
                        w = w2p.tile([128, D], F32R, tag="w2")
                        nc.sync.dma_start(w[:], w2t[ft])
                        for mt in range(MT):
                            for dc in range(2):
                                nc.tensor.matmul(
                                    pl2[mt * 2 + dc][:],
                                    h1T[:, ft, mt * 128:(mt + 1) * 128],
                                    w[:, dc * 512:(dc + 1) * 512],
                                    start=(ft == 0), stop=(ft == FT - 1))
                    for mt in range(MT):
                        o = otmp.tile([128, D], F32, tag="o")
                        for dc in range(2):
                            sl = slice(dc * 512, (dc + 1) * 512)
                            nc.vector.tensor_tensor(out=o[:, sl],
                                                    in0=pl2[mt * 2 + dc][:],
                                                    in1=resid[:, mt, sl], op=ALU.add)
                            nc.vector.tensor_tensor(
                                out=o[:, sl], in0=o[:, sl],
                                in1=b2_bc[:, sl],
                                op=ALU.add)
                        nc.sync.dma_start(out_out[mt * 128:(mt + 1) * 128, :], o[:])

    nc.compile()
    return nc


def _tile_w_et(w):
    # [D, E] -> [E/128, 128(p of d-tile), DT, 128] chunks contiguous per e-tile
    return np.ascontiguousarray(
        w.reshape(DT, 128, ET, 128).transpose(2, 1, 0, 3))


def _tile_w_ec(w, dt=np.float32):
    # [D, E] -> [2, 128, DT, 512]
    return np.ascontiguousarray(
        w.reshape(DT, 128, 2, 512).transpose(2, 1, 0, 3).astype(dt))


def _prep(inputs):
    f32 = np.float32
    x = np.asarray(inputs["x"], f32)
    q = np.asarray(inputs["q"], f32)
    w_q = np.asarray(inputs["w_q"], f32)
    w_k = np.asarray(inputs["w_k"], f32)
    w_v = np.asarray(inputs["w_v"], f32)
    w_o = np.asarray(inputs["w_o"], f32)
    ln2_g = np.asarray(inputs["ln2_g"], f32)
    ln2_b = np.asarray(inputs["ln2_b"], f32)
    w1 = np.asarray(inputs["w1"], f32)
    w2 = np.asarray(inputs["w2"], f32)

    shared = {}
    shared["wqt"] = _tile_w_et(w_q.T)
    shared["wkt"] = _tile_w_et(w_k.T)
    shared["wvt"] = _tile_w_ec(w_v.T)
    shared["wot"] = _tile_w_ec(w_o.T, ml_dtypes.bfloat16)
    w1p = (w1 * ln2_g[None, :]).T          # [d, f]
    shared["w1t"] = np.ascontiguousarray(
        w1p.reshape(DT, 128, FT, 128).transpose(2, 1, 0, 3))
    shared["w2t"] = np.ascontiguousarray(w2.T.reshape(FT, 128, D))
    shared["bq"] = np.ascontiguousarray(inputs["b_q"].reshape(ET, 128).T.astype(f32))
    shared["bk"] = np.ascontiguousarray(inputs["b_k"].reshape(ET, 128).T.astype(f32))
    shared["bv"] = np.asarray(inputs["b_v"], f32).reshape(1, D)
    shared["bo"] = np.asarray(inputs["b_o"], f32).reshape(1, D)
    b1p = np.asarray(inputs["b1"], f32) + w1 @ ln2_b
    shared["b1"] = np.ascontiguousarray(b1p.reshape(FT, 128).T.astype(f32))
    shared["b2"] = np.asarray(inputs["b2"], f32).reshape(1, D)

    in_maps = []
    for c in range(N_CORES):
        b = c // 2
        ms = (c % 2) * M
        m = dict(shared)
        m["xt"] = np.ascontiguousarray(
            x[b].T.reshape(DT, 128, N).transpose(1, 0, 2))
        qs = q[b, ms:ms + M]
        m["qt"] = np.ascontiguousarray(
            qs.T.reshape(DT, 128, M).transpose(1, 0, 2))
        m["q_in"] = np.ascontiguousarray(qs)
        in_maps.append(m)
    return in_maps


def kernel(**inputs):
    if "nc" not in _CACHE:
        _CACHE["nc"] = _build()
    nc = _CACHE["nc"]
    in_maps = _prep(inputs)
    res = run_bass_kernel_spmd(nc, in_maps, core_ids=list(range(N_CORES)))
    out = np.zeros((B, M_FULL, D), np.float32)
    attn = np.zeros((H, B, M_FULL, N), np.float32)
    for c in range(N_CORES):
        b = c // 2
        ms = (c % 2) * M
        out[b, ms:ms + M] = res.results[c]["out_out"]
        attn[:, b, ms:ms + M, :] = res.results[c]["attn_out"]
    return (out, attn)
